# revision 22
# baseline (speedup 1.0000x reference)
"""Capsule-routing kernel for Trainium2 (8 NeuronCores, data-parallel batch).

v2 design: per-batch software pipeline that hides nearly all compute under
the streaming input DMA.

  - u is cast-loaded fp32->bf16 by gpsimd DMA (no DVE/ACT convert pass).
  - All big transposes (u^T for the b-update, target^T, W^T, mask^T) run on
    the DMA XBAR (dma_start_transpose), not the PE.
  - v-matmul keeps u as the PE stationary operand (FWL bf16: 64cy/tile)
    with tiny 16-col rhs streams, so weight ingest is the only PE cost.
  - s is computed transposed (sT = W^T v per 128-chunk) so the squash norm
    comes from a tiny Gram matmul (obd^T obd diag) instead of wide 16-row
    DVE/ACT ops; rinv is folded into wo / the final output extraction.
  - Each batch's 3 routing iterations start as soon as its u slice lands;
    batch k+1 streams in while batch k routes.

Reference math (per batch):
    u_hat = u @ W;  b=0
    iter: c = softmax_n(b); t1 = target*c; s = t1 @ u_hat (block-diag)
          out = s/|s|; b = out . u_hat
    with u_hat never materialized:
      v[n,e] = sum_i t1[n,i] u[i,e];  sT[f, n] = sum_e W[e,f] v[n,e]
      WO[n,e] = sum_f out[n,f-block] W[e,f];  b[i,n] = sum_e u[i,e] WO[n,e]
"""

import sys

if "/opt/trn_rl_repo" not in sys.path:
    sys.path.insert(0, "/opt/trn_rl_repo")

import numpy as np

import concourse.bacc as bacc
import concourse.bass as bass
import concourse.tile as tile
from concourse import mybir
from concourse.masks import make_identity

F32 = mybir.dt.float32
BF16 = mybir.dt.bfloat16
P = 128


def _pin_act_tables():
    # Exp/Ln (softmax + squash rsqrt) both live in natural_log_exp_and_others;
    # strip them from other table sets so the load-insertion pass never
    # thrashes ~1.3us table loads between sets.
    import concourse.bacc as _bacc

    _orig = _bacc.get_activation_tables
    if getattr(_orig, "_act_pin", False):
        return
    _PIN = {"Exp", "Ln", "Square", "Copy", "Identity"}

    def _patched(arch):
        tables = _orig(arch)
        combined = "natural_log_exp_and_others"
        if combined not in tables:
            return tables
        pin = {f for f in tables[combined] if f.name in _PIN}
        return {
            name: (fns if name == combined else fns - pin)
            for name, fns in tables.items()
        }

    _patched._act_pin = True
    _bacc.get_activation_tables = _patched


_pin_act_tables()

B_LOC = 4       # batch elements per core (32 / 8 cores)
IN = 2048       # input capsules
DIN = 256       # input capsule dim (2 chunks of 128)
NCAP = 16
DCAP = 32
T = IN // P     # 16 i-tiles
NJ = DIN // P   # 2 e-chunks
NQ = 4          # f=512 -> 4 chunks of 128
F = NCAP * DCAP
EPS = 1e-7


def build_body(tc, o_ap, u_ap, tg_ap, w_ap):
    from contextlib import ExitStack

    nc = tc.nc
    ctx = ExitStack()

    const = ctx.enter_context(tc.tile_pool(name="const", bufs=1))
    sb_big = ctx.enter_context(tc.tile_pool(name="big", bufs=1))
    sb_tg = ctx.enter_context(tc.tile_pool(name="tg", bufs=4))
    work = ctx.enter_context(tc.tile_pool(name="work", bufs=6))
    small = ctx.enter_context(tc.tile_pool(name="small", bufs=10))
    ps_v = ctx.enter_context(tc.tile_pool(name="psv", bufs=1, space="PSUM"))
    ps_sT = ctx.enter_context(tc.tile_pool(name="psst", bufs=1, space="PSUM"))
    ps_g = ctx.enter_context(tc.tile_pool(name="psg", bufs=1, space="PSUM"))
    ps_wo = ctx.enter_context(tc.tile_pool(name="pswo", bufs=1, space="PSUM"))
    ps_bt = ctx.enter_context(tc.tile_pool(name="psbt", bufs=1, space="PSUM"))
    ps_wot = ctx.enter_context(tc.tile_pool(name="pswot", bufs=1, space="PSUM"))
    ps_T = ctx.enter_context(tc.tile_pool(name="psT", bufs=2, space="PSUM"))

    # ---- the input DMA stream first: gpsimd cast-DMAs issue in this order
    # and everything else pipelines behind it.  No XBAR transposes anywhere:
    # a DMA_TRANSPOSE needs exclusive queue access (full drain before+after),
    # which would serialize against the cast stream. ----
    # identities first: make_identity runs on the in-order gpsimd engine,
    # and every PE transpose waits on it
    identity_bf = const.tile([P, P], BF16)
    make_identity(nc, identity_bf)
    ident16f = const.tile([NCAP, NCAP], F32)
    make_identity(nc, ident16f)

    u16_sb, ut2_sb, tt_sb, tg_sb = [], [], [], []
    for b in range(B_LOC):
        tg16 = sb_tg.tile([NCAP, IN], BF16, tag="tg16")
        nc.gpsimd.dma_start(out=tg16, in_=tg_ap[b])
        tg_sb.append(tg16)

    # W + mask ahead of the u stream: both gate the first batch's tail
    # (sT needs w16/wtq, obd needs maskT); ~1.5us of u delay buys ~14us
    # earlier chain starts.
    w16 = const.tile([P, NJ, F], BF16)
    nc.gpsimd.dma_start(out=w16, in_=w_ap.rearrange("(j p) f -> p j f", p=P))
    mask16f = const.tile([NCAP, F], F32)
    nc.gpsimd.memset(mask16f, 0.0)
    nc.gpsimd.affine_select(
        out=mask16f.rearrange("p (a b) -> p a b", b=DCAP),
        in_=mask16f.rearrange("p (a b) -> p a b", b=DCAP),
        compare_op=mybir.AluOpType.not_equal,
        fill=1.0,
        base=0,
        pattern=[[-1, NCAP], [0, DCAP]],
        channel_multiplier=1,
    )

    for b in range(B_LOC):
        u16 = sb_big.tile([P, T, DIN], BF16, tag=f"u{b}")
        src = u_ap[b].rearrange("(t p) e -> p t e", p=P)
        for h in range(2):
            sl = slice(h * (T // 2), (h + 1) * (T // 2))
            nc.gpsimd.dma_start(out=u16[:, sl, :], in_=src[:, sl, :])
        u16_sb.append(u16)
        ut2 = sb_big.tile([P, T, NJ, P], BF16, tag=f"ut{b}")
        ut2_sb.append(ut2)

    ones_row = const.tile([1, P], F32)
    nc.vector.memset(ones_row, 1.0)

    eps0 = const.tile([NCAP, 1], F32)
    nc.vector.memset(eps0, EPS * (NCAP * NCAP))
    eps1 = const.tile([NCAP, 1], F32)
    nc.vector.memset(eps1, EPS)

    # tt[p, t, n] = target[b, n, t*128+p] via bf16 PE transposes (cheap
    # FWL weight loads; runs early while u streams in, warming the PE)
    for b in range(B_LOC):
        stg = ps_T.tile([P, T, NCAP], BF16, tag="T")
        for t in range(T):
            nc.tensor.transpose(stg[:, t, :], tg_sb[b][:, t * P:(t + 1) * P],
                                identity_bf[:NCAP, :NCAP])
        tt = sb_big.tile([P, T, NCAP], BF16, tag=f"tt{b}")
        nc.scalar.copy(out=tt.rearrange("p a b -> p (a b)"),
                       in_=stg.rearrange("p a b -> p (a b)"))
        tt_sb.append(tt)

    # wtq[p, j, q, m] = W[j*128+m, q*128+p]  (rows = f-chunk q, cols = e)
    wtq = const.tile([P, NJ, NQ, P], BF16)
    for j in range(NJ):
        psW = ps_T.tile([P, NQ, P], BF16, tag="T")
        for q in range(NQ):
            nc.tensor.transpose(psW[:, q, :], w16[:, j, q * P:(q + 1) * P],
                                identity_bf)
        nc.vector.tensor_copy(out=wtq[:, j].rearrange("p a b -> p (a b)"),
                              in_=psW.rearrange("p a b -> p (a b)"))

    # block-diag mask: mask16[n, (n',d)] = 1 iff n'==n ; maskT = its PE^T
    mask16 = const.tile([NCAP, F], BF16)
    nc.vector.tensor_copy(out=mask16, in_=mask16f)
    psM = ps_T.tile([P, NQ, NCAP], BF16, tag="T")
    for q in range(NQ):
        nc.tensor.transpose(psM[:, q, :], mask16[:, q * P:(q + 1) * P],
                            identity_bf[:NCAP, :NCAP])
    maskT = const.tile([P, NQ, NCAP], BF16)
    nc.vector.tensor_copy(out=maskT.rearrange("p a b -> p (a b)"),
                          in_=psM.rearrange("p a b -> p (a b)"))

    def v0mm(b):
        # iter-0 v-matmul + the u^T build, as two long uniform PE streams
        # (keeps the PE gap-free so it ramps to full clock). PSUM transposes
        # land in SBUF ut2 via copies alternating DVE/ACT.
        ut2 = ut2_sb[b]
        v_ps = ps_v.tile([P, NJ, NCAP], F32, tag="v")
        for j in range(NJ):
            for t in range(T):
                nc.tensor.matmul(
                    v_ps[:, j, :],
                    lhsT=u16_sb[b][:, t, j * P:(j + 1) * P],
                    rhs=tt_sb[b][:, t, :],
                    start=(t == 0), stop=(t == T - 1),
                )
        vt = work.tile([P, NJ, NCAP], BF16, tag="vt")
        nc.vector.tensor_copy(out=vt.rearrange("p a b -> p (a b)"),
                              in_=v_ps.rearrange("p a b -> p (a b)"))
        cnt = 0
        for j in range(NJ):
            for tq in range(4):
                psT = ps_T.tile([P, 4, P], BF16, tag="T")
                for k in range(4):
                    t = 4 * tq + k
                    nc.tensor.transpose(
                        psT[:, k, :], u16_sb[b][:, t, j * P:(j + 1) * P],
                        identity_bf)
                cnt += 1
                out = ut2[:, 4 * tq:4 * tq + 4, j, :]
                if cnt % 4 == 0:
                    nc.vector.tensor_copy(out=out, in_=psT)
                else:
                    nc.scalar.copy(out=out, in_=psT)
        return vt

    def vmm(b, t1):
        # vT[e(=p), j, n] = sum_i u[i, e] t1[i, n]   (u tiles stationary)
        v_ps = ps_v.tile([P, NJ, NCAP], F32, tag="v")
        for j in range(NJ):
            for t in range(T):
                nc.tensor.matmul(
                    v_ps[:, j, :],
                    lhsT=u16_sb[b][:, t, j * P:(j + 1) * P],
                    rhs=t1[:, t, :],
                    start=(t == 0),
                    stop=(t == T - 1),
                )
        vt = work.tile([P, NJ, NCAP], BF16, tag="vt")
        nc.vector.tensor_copy(out=vt.rearrange("p a b -> p (a b)"),
                              in_=v_ps.rearrange("p a b -> p (a b)"))
        return vt

    def tail(b, it, vt):
        """sT -> obd -> (gram->ss->rinv) ; wo/wot (it<2) or output (it==2).
        Returns t1 for the next iteration (None when it==2)."""
        # sT[f-chunk q rows, n] = sum_e W[e, f] v[n, e]
        sT_ps = ps_sT.tile([P, NQ, NCAP], F32, tag="st")
        for q in range(NQ):
            for j in range(NJ):
                nc.tensor.matmul(
                    sT_ps[:, q, :],
                    lhsT=w16[:, j, q * P:(q + 1) * P],
                    rhs=vt[:, j, :],
                    start=(j == 0),
                    stop=(j == NJ - 1),
                )
        # keep only block-diag rows: obd[p, q, n] = sT * maskT  (bf16)
        obd = work.tile([P, NQ, NCAP], BF16, tag="obd")
        nc.vector.tensor_tensor(
            out=obd.rearrange("p a b -> p (a b)"),
            in0=sT_ps.rearrange("p a b -> p (a b)"),
            in1=maskT.rearrange("p a b -> p (a b)"),
            op=mybir.AluOpType.mult,
        )
        # ||s_n||^2 via Gram diag: gram = sum_q obd_q^T obd_q
        gram_ps = ps_g.tile([NCAP, NCAP], F32, tag="g")
        for q in range(NQ):
            nc.tensor.matmul(
                gram_ps,
                lhsT=obd[:, q, :],
                rhs=obd[:, q, :],
                start=(q == 0),
                stop=(q == NQ - 1),
            )
        diag = small.tile([NCAP, NCAP], F32, tag="junk")
        nc.vector.tensor_mul(diag, gram_ps, identity_bf[:NCAP, :NCAP])
        ss = small.tile([NCAP, 1], F32, tag="ss")
        nc.vector.reduce_sum(ss, diag, axis=mybir.AxisListType.X)
        # rinv = (ss+eps)^-0.5 via exp(-0.5*ln(ss+eps)) (stays on exp/ln table)
        lnv = small.tile([NCAP, 1], F32, tag="lnv")
        nc.scalar.activation(lnv, ss, mybir.ActivationFunctionType.Ln,
                             bias=(eps0 if it == 0 else eps1))
        rinv = small.tile([NCAP, 1], F32, tag="rinv")
        nc.scalar.activation(rinv, lnv, mybir.ActivationFunctionType.Exp,
                             scale=-0.5)

        if it == 2:
            # outputs: out[n, d] = rinv[n] * sT[(n,d), n] ; transpose the
            # masked chunks back to [n, (nd)] rows and reduce over n'
            m_ps = ps_wo.tile([NCAP, NQ, P], BF16, tag="wo")
            for q in range(NQ):
                nc.tensor.transpose(m_ps[:, q, :], obd[:, q, :], identity_bf)
            outp_m = small.tile([NCAP, DCAP], F32, tag="outm")
            nc.vector.reduce_sum(
                outp_m,
                bass.AP(
                    tensor=m_ps.tensor,
                    offset=m_ps.offset,
                    ap=[m_ps.ap[0], [1, DCAP], [DCAP, NCAP]],
                ),
                axis=mybir.AxisListType.X,
            )
            outp = small.tile([NCAP, DCAP], F32, tag="outp")
            nc.vector.tensor_scalar_mul(outp, outp_m, rinv)
            nc.sync.dma_start(out=o_ap[b], in_=outp)
            return None

        # WO^T[e, n] = sum_{f} W[e, f] obd[f, n]  -- directly in wot layout
        wot_ps = ps_wot.tile([P, NJ, NCAP], F32, tag="wot")
        for j in range(NJ):
            for q in range(NQ):
                nc.tensor.matmul(
                    wot_ps[:, j, :],
                    lhsT=wtq[:, j, q, :],
                    rhs=obd[:, q, :],
                    start=(q == 0),
                    stop=(q == NQ - 1),
                )
        # rinv broadcast to all 128 partitions: rinvT = rinv^T (1x16) via a
        # K=16 matmul against I16, then ones (x) rinvT
        rT_ps = ps_g.tile([1, NCAP], F32, tag="g")
        nc.tensor.matmul(rT_ps, lhsT=rinv, rhs=ident16f)
        rT = small.tile([1, NCAP], F32, tag="rT")
        nc.vector.tensor_copy(out=rT, in_=rT_ps)
        rbc_ps = ps_g.tile([P, NCAP], F32, tag="g")
        nc.tensor.matmul(rbc_ps, lhsT=ones_row, rhs=rT)
        rbc_sb = small.tile([P, NCAP], F32, tag="rbc")
        nc.vector.tensor_copy(out=rbc_sb, in_=rbc_ps)
        wot = work.tile([P, NJ, NCAP], BF16, tag="wot")
        rbc = bass.AP(
            tensor=rbc_sb.tensor,
            offset=rbc_sb.offset,
            ap=[rbc_sb.ap[0], [0, NJ], [1, NCAP]],
        )
        nc.vector.tensor_tensor(
            out=wot, in0=wot_ps, in1=rbc, op=mybir.AluOpType.mult)

        # b-update: bT[i(=p), t, n] = sum_e u[i, e] WO[n, e]
        bt_ps = ps_bt.tile([P, T, NCAP], F32, tag="bt")
        for t in range(T):
            for j in range(NJ):
                nc.tensor.matmul(
                    bt_ps[:, t, :],
                    lhsT=ut2_sb[b][:, t, j, :],
                    rhs=wot[:, j, :],
                    start=(j == 0),
                    stop=(j == NJ - 1),
                )
        # softmax over n + target modulation -> t1 (bf16)
        e_sb = work.tile([P, T, NCAP], F32, tag="esb")
        nc.scalar.activation(e_sb.rearrange("p a b -> p (a b)"),
                             bt_ps.rearrange("p a b -> p (a b)"),
                             mybir.ActivationFunctionType.Exp)
        den = small.tile([P, T], F32, tag="den")
        nc.vector.reduce_sum(den, e_sb, axis=mybir.AxisListType.X)
        rden = small.tile([P, T], F32, tag="rden")
        nc.vector.reciprocal(rden, den)
        tmp = work.tile([P, T, NCAP], F32, tag="tmp")
        nc.vector.tensor_mul(tmp.rearrange("p a b -> p (a b)"),
                             e_sb.rearrange("p a b -> p (a b)"),
                             tt_sb[b].rearrange("p a b -> p (a b)"))
        t1 = work.tile([P, T, NCAP], BF16, tag="t1")
        rden_bc = bass.AP(
            tensor=rden.tensor,
            offset=rden.offset,
            ap=[rden.ap[0], [1, T], [0, NCAP]],
        )
        nc.vector.tensor_tensor(
            out=t1, in0=tmp, in1=rden_bc, op=mybir.AluOpType.mult)
        return t1

    # Stage-interleaved emission: batch k+1's prologue fills batch k's
    # softmax/squash chain stalls on the PE.  (iter 0: c uniform -> raw
    # target as t1, scale folded into eps0.)
    t1s = {}
    vt0 = {}
    vt0[0] = v0mm(0)
    t1s[0] = tail(0, 0, vt0[0])
    vt0[1] = v0mm(1)
    t1s[1] = tail(1, 0, vt0[1])
    t1s[0] = tail(0, 1, vmm(0, t1s[0]))
    vt0[2] = v0mm(2)
    t1s[2] = tail(2, 0, vt0[2])
    t1s[1] = tail(1, 1, vmm(1, t1s[1]))
    tail(0, 2, vmm(0, t1s[0]))
    vt0[3] = v0mm(3)
    t1s[3] = tail(3, 0, vt0[3])
    t1s[2] = tail(2, 1, vmm(2, t1s[2]))
    tail(1, 2, vmm(1, t1s[1]))
    t1s[3] = tail(3, 1, vmm(3, t1s[3]))
    tail(2, 2, vmm(2, t1s[2]))
    tail(3, 2, vmm(3, t1s[3]))

    ctx.close()


def build_nc(loop_n=0):
    nc = bacc.Bacc("TRN2")
    u = nc.dram_tensor("u_vecs", [B_LOC, IN, DIN], F32, kind="ExternalInput").ap()
    tg = nc.dram_tensor("target", [B_LOC, NCAP, IN], F32, kind="ExternalInput").ap()
    w = nc.dram_tensor("W", [DIN, F], F32, kind="ExternalInput").ap()
    o = nc.dram_tensor("out", [B_LOC, NCAP, DCAP], F32, kind="ExternalOutput").ap()
    with tile.TileContext(nc) as tc:
        if loop_n:
            with tc.For_i(0, loop_n, 1):
                build_body(tc, o, u, tg, w)
        else:
            build_body(tc, o, u, tg, w)
    nc.compile()
    return nc


_NC_CACHE = None


def kernel(u_vecs, target, W, _trace=False, **_trace_kwargs):
    global _NC_CACHE
    from concourse.bass_utils import run_bass_kernel_spmd

    if _NC_CACHE is None:
        _NC_CACHE = build_nc()
    nc = _NC_CACHE

    n_cores = 8
    in_maps = []
    for c in range(n_cores):
        sl = slice(c * B_LOC, (c + 1) * B_LOC)
        in_maps.append({
            "u_vecs": np.ascontiguousarray(u_vecs[sl]),
            "target": np.ascontiguousarray(target[sl]),
            "W": np.ascontiguousarray(W),
        })
    res = run_bass_kernel_spmd(nc, in_maps, list(range(n_cores)),
                               trace=_trace, **_trace_kwargs)
    out = np.concatenate([res.results[c]["out"] for c in range(n_cores)], axis=0)
    if _trace:
        return out, res
    return out


if __name__ == "__main__":
    rng = np.random.default_rng(0)
    u = rng.standard_normal((32, IN, DIN), dtype=np.float32)
    t = rng.random((32, NCAP, IN), dtype=np.float32)
    w = (rng.standard_normal((DIN, F)) * 0.06).astype(np.float32)
    print(kernel(u, t, w).shape)
